# revision 1
# baseline (speedup 1.0000x reference)
"""Trainium2 Bass kernel for nn_EnhancedAttentionLayer (GAT-style masked attention).

Data-parallel over batch: B=8 batch elements -> 8 NeuronCores, one each.
Params replicated. No collectives.

Math (per batch element, all heads on one core):
  h{1,2,3} = feat @ W{1,2,3}[h]      (per-head projections)
  t1 = tanh(h1); src = t1 @ w_src; dst = t1 @ w_dst
  attn1[i,j] = leaky_relu(src[i] + dst[j], 0.2)
  p2 = exp(attn1) * m2, p3 = exp(attn1) * m3   (masks from s_mask / adj; exp
        without max-subtraction is safe here: |attn1| <= ~50)
  out2 = (p2 @ h2) / rowsum(p2)      (denominator via ones-column in rhs)
  feat_out = (diag(adj)*h1 + out2 + out3 + b)/3 ; elu ; sigmoid-gated residual

Key layouts:
  - p built TRANSPOSED [j, i] so it feeds matmul lhsT directly (no [N,N]
    transposes on device; adj/s_mask arrive host-transposed).
  - exp(leaky(x)) = max(exp(x), exp(0.2x)); exp(src+dst) = exp(src)*exp(dst)
    is rank-1, so only per-vector exps on ACT + outer products via cheap
    tensor_scalar ops.
  - f32r (TF32-like, full PE rate at N>=256) for the h1/score path;
    bf16 for h2/h3/gate matmuls and the [N,N] elementwise stage.
"""

import numpy as np
import ml_dtypes

import concourse.bass as bass
import concourse.tile as tile
from concourse import bacc, mybir
from concourse.bass_utils import run_bass_kernel_spmd

F32 = mybir.dt.float32
F32R = mybir.dt.float32r
BF16 = mybir.dt.bfloat16
I32 = mybir.dt.int32
AF = mybir.ActivationFunctionType
OP = mybir.AluOpType

B, N, D = 8, 512, 768
H, E = 8, 96
IC = N // 128   # 4 i-chunks (attention rows / output nodes)
JC = N // 128   # 4 j-chunks (attention cols / neighbor nodes)
DC = D // 128   # 6 contraction chunks
EPS = 1e-30

_CACHED = None


def _r(ap):
    return ap.bitcast(F32R)


def build_kernel(with_bias: bool):
    nc = bacc.Bacc("TRN2", target_bir_lowering=False, debug=False, num_devices=B)

    # ---- per-core DRAM tensors (host pre-laid-out; see kernel() below) ----
    feat_n = nc.dram_tensor("feat_n", [128, IC * D], F32, kind="ExternalInput").ap()
    featT = nc.dram_tensor("featT", [128, DC * N], BF16, kind="ExternalInput").ap()
    adjT_i = nc.dram_tensor("adjT_i", [128, JC * N], BF16, kind="ExternalInput").ap()
    smT_i = nc.dram_tensor("smT_i", [128, JC * N], BF16, kind="ExternalInput").ap()
    adjd = nc.dram_tensor("adjd", [128, IC], F32, kind="ExternalInput").ap()
    W1p = nc.dram_tensor("W1p", [128, DC * H * E], BF16, kind="ExternalInput").ap()
    W23p = nc.dram_tensor("W23p", [128, DC * H * 2 * E], BF16, kind="ExternalInput").ap()
    wsd = nc.dram_tensor("wsd", [96, 16], F32, kind="ExternalInput").ap()
    Hwt = nc.dram_tensor("Hwt", [128, DC * D], BF16, kind="ExternalInput").ap()
    Hb = nc.dram_tensor("Hb", [1, D], BF16, kind="ExternalInput").ap()
    omeye = nc.dram_tensor("omeye", [128, JC * N], BF16, kind="ExternalInput").ap()
    eye128 = nc.dram_tensor("eye128", [128, 128], BF16, kind="ExternalInput").ap()
    ones_row = nc.dram_tensor("ones_row", [1, 512], BF16, kind="ExternalInput").ap()
    ones128 = nc.dram_tensor("ones128", [128, 128], F32, kind="ExternalInput").ap()
    if with_bias:
        b3row = nc.dram_tensor("b3row", [1, D], F32, kind="ExternalInput").ap()
    out = nc.dram_tensor("out", [N, D], F32, kind="ExternalOutput").ap()

    with tile.TileContext(nc) as tc:
        with tc.tile_pool(name="persist", bufs=1) as P:
            # ------------- persistent SBUF tiles (live whole kernel) -------
            adjd_sb = P.tile([128, IC], F32, tag="adjd_sb")
            adjd3_sb = P.tile([128, IC], F32, tag="adjd3_sb")
            Hb_sb = P.tile([1, D], BF16, tag="Hb_sb")
            eye_sb = P.tile([128, 128], BF16, tag="eye_sb")
            onesr_sb = P.tile([1, 512], BF16, tag="onesr_sb")
            ones_sb = P.tile([128, 128], F32, tag="ones_sb")
            m2T = P.tile([128, JC * N], BF16, tag="m2T")               # 4K
            m3T = P.tile([128, JC * N], BF16, tag="m3T")               # 4K
            h23 = P.tile([128, JC * H * 2 * 97], BF16, tag="h23")      # 12.1K
            es_rep = P.tile([128, H * N], BF16, tag="es_rep")          # 8K
            es2_rep = P.tile([128, H * N], BF16, tag="es2_rep")        # 8K
            ed_sb = P.tile([128, H * JC], F32, tag="ed_sb")
            ed2_sb = P.tile([128, H * JC], F32, tag="ed2_sb")
            gate_sb = P.tile([128, IC * D], F32, tag="gate_sb")        # 12K
            out23 = P.tile([128, IC * D], F32, tag="out23")            # 12K
            intra = P.tile([128, IC * D], F32, tag="intra")            # 12K
            b3_sb = P.tile([128, D], F32, tag="b3_sb") if with_bias else None

            h23r = h23[:].rearrange("p (jc h k eo) -> p jc h k eo",
                                    jc=JC, h=H, k=2, eo=97)
            intrar = intra[:].rearrange("p (i h e) -> p i h e", i=IC, h=H, e=96)

            # ---------------- input DMAs (persistent) ----------------
            nc.sync.dma_start(adjd_sb[:], adjd)
            nc.vector.tensor_scalar(adjd3_sb[:], adjd_sb[:], 1.0 / 3.0, None,
                                    OP.mult)
            nc.sync.dma_start(Hb_sb[:], Hb)
            nc.sync.dma_start(eye_sb[:], eye128)
            nc.sync.dma_start(onesr_sb[:], ones_row)
            nc.sync.dma_start(ones_sb[:], ones128)

            PP = tc.alloc_tile_pool(name="ppool", bufs=4)
            AB = tc.alloc_tile_pool(name="abpool", bufs=3)
            EV = tc.alloc_tile_pool(name="evpool", bufs=3)
            with tc.tile_pool(name="wpool", bufs=1) as WP:
                W1p_sb = WP.tile([128, DC * H * E], BF16, tag="W1p_sb")      # 9K
                W23p_sb = WP.tile([128, DC * H * 2 * E], BF16, tag="W23p_sb")  # 18K
                Hwt_sb = WP.tile([128, DC * D], BF16, tag="Hwt_sb")          # 9K
                featT_bf = WP.tile([128, DC * N], BF16, tag="featT_bf")      # 6K
                TP = tc.alloc_tile_pool(name="tpool", bufs=1)
                t1T = TP.tile([96, H * N], BF16, tag="t1T")                  # 8K
                h1T = TP.tile([96, H * N], BF16, tag="h1T")                  # 8K
                wsr = TP.tile([96, H * 128], BF16, tag="wsr")                # 2K
                wsd_bf = TP.tile([96, 16], BF16, tag="wsd_bf")
                wsd_sb = TP.tile([96, 16], F32, tag="wsd_sb")
                MP = tc.alloc_tile_pool(name="mpool", bufs=1)
                omeye_sb = MP.tile([128, JC * N], BF16, tag="omeye_sb")      # 4K
                smT_bf = MP.tile([128, JC * N], BF16, tag="smT_bf")          # 4K
                adjT_bf = MP.tile([128, JC * N], BF16, tag="adjT_bf")        # 4K
                omsm = MP.tile([128, JC * N], BF16, tag="omsm")              # 4K

                nc.sync.dma_start(wsd_sb[:], wsd)
                for dc in range(DC):
                    nc.sync.dma_start(featT_bf[:, dc * N:(dc + 1) * N],
                                      featT[:, dc * N:(dc + 1) * N])
                    nc.sync.dma_start(
                        W1p_sb[:, dc * 768:(dc + 1) * 768],
                        W1p[:, dc * 768:(dc + 1) * 768])
                nc.sync.dma_start(smT_bf[:], smT_i)
                nc.sync.dma_start(adjT_bf[:], adjT_i)
                nc.sync.dma_start(omeye_sb[:], omeye)
                nc.sync.dma_start(W23p_sb[:], W23p)
                nc.sync.dma_start(Hwt_sb[:], Hwt)
                nc.vector.tensor_copy(wsd_bf[:], wsd_sb[:])

                # ---------------- masks (bf16 0/1) ----------------
                nc.vector.tensor_tensor(m2T[:], smT_bf[:], omeye_sb[:], OP.mult)
                nc.vector.tensor_scalar(omsm[:], smT_bf[:], -1.0, 1.0,
                                        OP.mult, OP.add)              # 1 - smT
                nc.vector.tensor_tensor(m3T[:], adjT_bf[:], omsm[:], OP.mult)
                MP.release()

                # ---------------- h1 (transposed [e, i]) + tanh -------------
                with tc.tile_pool(name="ps1p", bufs=2, space="PSUM") as PS1:
                    for h in range(H):
                        ps1 = PS1.tile([96, 512], F32, tag="ps1")
                        for dc in range(DC):
                            nc.tensor.matmul(
                                ps1[:, :],
                                W1p_sb[:, dc * 768 + h * 96:
                                       dc * 768 + (h + 1) * 96],
                                featT_bf[:, dc * N:(dc + 1) * N],
                                start=(dc == 0), stop=(dc == DC - 1))
                        nc.scalar.activation(t1T[0:96, h * N:(h + 1) * N],
                                             ps1[:, :], AF.Tanh)
                        nc.vector.tensor_copy(h1T[0:96, h * N:(h + 1) * N],
                                              ps1[:, :])

                # ------- interleaved src/dst exps + h2/h3 projections -------
                for h in range(H):
                    nc.vector.tensor_scalar(
                        wsr[0:96, h * 128:(h + 1) * 128], ones_sb[0:96, 0:128],
                        wsd_sb[0:96, h:h + 1], None, OP.mult)
                nc.vector.memset(h23r[:, :, :, :, 96:97], 1.0)        # ones cols
                with (
                    tc.tile_pool(name="pssp", bufs=1, space="PSUM") as PSS,
                    tc.tile_pool(name="ps23p", bufs=1, space="PSUM") as PS23,
                    tc.tile_pool(name="psgp", bufs=1, space="PSUM") as PSG,
                ):
                    for s in range(IC):
                        if s % 2 == 0:
                            psd = PSS.tile([128, 16], F32, tag="psd", bufs=1,
                                           name=f"psd_{s}")
                        for h in (2 * s, 2 * s + 1):
                            pss = PSS.tile([128, 512], F32, tag="pss")
                            nc.tensor.matmul(pss[:, :],
                                             wsr[0:96, h * 128:(h + 1) * 128],
                                             t1T[0:96, h * N:(h + 1) * N],
                                             start=True, stop=True)
                            nc.scalar.activation(es_rep[:, h * N:(h + 1) * N],
                                                 pss[:, :], AF.Exp)
                            nc.scalar.activation(es2_rep[:, h * N:(h + 1) * N],
                                                 pss[:, :], AF.Exp, scale=0.2)
                            for jc in range(JC):
                                nc.tensor.matmul(
                                    psd[:, (h % 4) * JC + jc:
                                        (h % 4) * JC + jc + 1],
                                    t1T[0:96, h * N + jc * 128:
                                        h * N + (jc + 1) * 128],
                                    wsd_bf[0:96, 8 + h:9 + h],
                                    start=True, stop=True)
                        if s % 2 == 1:
                            g0 = (s - 1) * 2 * JC
                            nc.scalar.activation(
                                ed_sb[:, g0:g0 + 16], psd[:, :], AF.Exp)
                            nc.scalar.activation(
                                ed2_sb[:, g0:g0 + 16], psd[:, :], AF.Exp,
                                scale=0.2)
                        mc = s
                        ps23 = PS23.tile([128, 2048], F32, tag="ps23")
                        for dc in range(DC):
                            lhsT = featT_bf[:, dc * N + mc * 128:
                                            dc * N + (mc + 1) * 128]
                            for g in range(4):
                                nc.tensor.matmul(
                                    ps23[:, g * 512:g * 512 + 384],
                                    lhsT,
                                    W23p_sb[:, dc * 1536 + g * 384:
                                            dc * 1536 + (g + 1) * 384],
                                    start=(dc == 0), stop=(dc == DC - 1))
                        for g in range(4):
                            psrc = ps23[:, g * 512:g * 512 + 384].rearrange(
                                "p (h k e) -> p h k e", h=2, k=2, e=96)
                            eng = nc.scalar.copy if g % 2 == 0 else \
                                nc.vector.tensor_copy
                            eng(h23r[:, mc, 2 * g:2 * g + 2, :, 0:96], psrc)
                        psg = PSG.tile([128, 1024], F32, tag="psg")
                        for dc in range(DC):
                            lhsT = featT_bf[:, dc * N + mc * 128:
                                            dc * N + (mc + 1) * 128]
                            nc.tensor.matmul(psg[:, 0:512], lhsT,
                                             Hwt_sb[:, dc * D:dc * D + 512],
                                             start=(dc == 0), stop=False)
                            nc.tensor.matmul(psg[:, 512:768], lhsT,
                                             Hwt_sb[:, dc * D + 512:(dc + 1) * D],
                                             start=(dc == 0), stop=False)
                        nc.tensor.matmul(psg[:, 0:512], onesr_sb[0:1, 0:128],
                                         Hb_sb[0:1, 0:512],
                                         start=False, stop=True)
                        nc.tensor.matmul(psg[:, 512:768], onesr_sb[0:1, 0:128],
                                         Hb_sb[0:1, 512:768],
                                         start=False, stop=True)
                        nc.scalar.activation(gate_sb[:, mc * D:(mc + 1) * D],
                                             psg[:, 0:768], AF.Sigmoid)

                # ---------------- h1 transpose + intra term -----------------
                with tc.tile_pool(name="pstp", bufs=2, space="PSUM") as PST:
                    for ic in range(IC):
                        pst = PST.tile([128, 1024], BF16, tag="pst")
                        for h in range(H):
                            nc.tensor.transpose(
                                pst[:, h * 128:h * 128 + 96],
                                h1T[0:96, h * N + ic * 128:h * N + (ic + 1) * 128],
                                eye_sb[0:96, 0:96])
                        pstr = pst[:].rearrange("p (u h o) -> p u h o",
                                                u=1, h=H, o=128)
                        nc.scalar.activation(
                            intrar[:, ic:ic + 1], pstr[:, :, :, 0:96],
                            AF.Copy, scale=adjd3_sb[:, ic:ic + 1])
                TP.release()

                if with_bias:
                    with tc.tile_pool(name="psbp", bufs=1, space="PSUM") as PSB:
                        psb = PSB.tile([128, D], F32, tag="psb")
                        b3d = WP.tile([1, D], BF16, tag="b3d")
                        nc.gpsimd.dma_start(b3d[:], b3row)
                        nc.tensor.matmul(psb[:, 0:512], onesr_sb[0:1, 0:128],
                                         b3d[0:1, 0:512], start=True, stop=True)
                        nc.tensor.matmul(psb[:, 512:768], onesr_sb[0:1, 0:128],
                                         b3d[0:1, 512:768], start=True, stop=True)
                        nc.vector.tensor_copy(b3_sb[:], psb[:, :])

            # ---------------- attention rounds ----------------
            with (
                tc.tile_pool(name="psrp", bufs=8, space="PSUM") as PSR,
                tc.tile_pool(name="fpool", bufs=3) as FP,
            ):
                p2t = {}
                p3t = {}

                def build_head(h, pool_heavy):
                    p2 = PP.tile([128, JC * N], BF16, tag="p2",
                                 name=f"p2_h{h}")
                    p3 = PP.tile([128, JC * N], BF16, tag="p3",
                                 name=f"p3_h{h}")
                    p2t[h], p3t[h] = p2, p3
                    for jc in range(JC):
                        a_t = AB.tile([128, N], BF16, tag="a_t",
                                      name=f"a_{h}_{jc}")
                        b_t = AB.tile([128, N], BF16, tag="b_t",
                                      name=f"b_{h}_{jc}")
                        e_t = AB.tile([128, N], BF16, tag="e_t",
                                      name=f"e_{h}_{jc}")
                        nc.vector.tensor_scalar(
                            a_t[:], es_rep[:, h * N:(h + 1) * N],
                            ed_sb[:, h * JC + jc:h * JC + jc + 1],
                            None, OP.mult)
                        nc.vector.tensor_scalar(
                            b_t[:], es2_rep[:, h * N:(h + 1) * N],
                            ed2_sb[:, h * JC + jc:h * JC + jc + 1],
                            None, OP.mult)
                        nc.vector.tensor_tensor(e_t[:], a_t[:], b_t[:],
                                                OP.max)
                        nc.vector.tensor_tensor(
                            p2[:, jc * N:(jc + 1) * N], e_t[:],
                            m2T[:, jc * N:(jc + 1) * N], OP.mult)
                        nc.gpsimd.tensor_tensor(
                            p3[:, jc * N:(jc + 1) * N], e_t[:],
                            m3T[:, jc * N:(jc + 1) * N], OP.mult)

                def mms_head(h, psa):
                    hh = h % 2
                    p2, p3 = p2t[h], p3t[h]
                    for ic in range(IC):
                        for k, p in ((0, p2), (1, p3)):
                            off = hh * 256 + k * 128
                            for jc in range(JC):
                                nc.tensor.matmul(
                                    psa[ic][:, off:off + 97],
                                    p[:, jc * N + ic * 128:
                                      jc * N + (ic + 1) * 128],
                                    h23[:, jc * 1552 + h * 194 + k * 97:
                                        jc * 1552 + h * 194 + k * 97 + 97],
                                    start=(jc == 0), stop=(jc == JC - 1))

                def evac_round(rnd, psa):
                    for ic in range(IC):
                        par = psa[ic][:].rearrange("p (s k o) -> p s k o",
                                                   s=2, k=2, o=128)
                        dden = EV.tile([128, 4], F32, tag="dden",
                                       name=f"dd_{rnd}_{ic}")
                        rcol = EV.tile([128, 4], F32, tag="rcol",
                                       name=f"rc_{rnd}_{ic}")
                        t23 = EV.tile([128, 384], F32, tag="t23",
                                      name=f"t23_{rnd}_{ic}")
                        ddenr = dden[:].rearrange("p (s k o) -> p s k o",
                                                  s=2, k=2, o=1)
                        nc.vector.tensor_scalar(
                            ddenr, par[:, :, :, 96:97], EPS, 3.0,
                            OP.add, OP.mult)
                        nc.vector.reciprocal(rcol[:], dden[:])
                        rbc = rcol[:].rearrange("p (s k) -> p s k", s=2, k=2) \
                                     .broadcast_to([128, 2, 2, 96])
                        t23r = t23[:].rearrange("p (s k e) -> p s k e",
                                                s=2, k=2, e=96)
                        nc.vector.tensor_tensor(t23r, par[:, :, :, 0:96],
                                                rbc, OP.mult)
                        nc.gpsimd.tensor_tensor(
                            out23[:, ic * D + rnd * 192:
                                  ic * D + rnd * 192 + 192]
                            .rearrange("p (s u e) -> p s u e", s=2, u=1, e=96),
                            t23r[:, :, 0:1, :], t23r[:, :, 1:2, :], OP.add)


                HD = 384
                def emit_final(hf):
                    for ic in range(IC):
                        lo = ic * D + hf * HD
                        fchunk = FP.tile([128, HD], F32, tag="fchunk",
                                         name=f"fch_{ic}_{hf}")
                        nc.sync.dma_start(fchunk[:], feat_n[:, lo:lo + HD])
                        pre = FP.tile([128, HD], F32, tag="pre",
                                      name=f"pre_{ic}_{hf}")
                        nc.vector.tensor_tensor(pre[:], out23[:, lo:lo + HD],
                                                intra[:, lo:lo + HD], OP.add)
                        if with_bias:
                            nc.gpsimd.tensor_tensor(
                                pre[:], pre[:],
                                b3_sb[:, hf * HD:(hf + 1) * HD], OP.add)
                        e1 = FP.tile([128, HD], F32, tag="e1",
                                     name=f"e1_{ic}_{hf}")
                        nc.scalar.activation(e1[:], pre[:], AF.Exp)
                        # em = relu(1 - e1) = -min(e1 - 1, 0)
                        em = FP.tile([128, HD], BF16, tag="em",
                                     name=f"em_{ic}_{hf}")
                        nc.scalar.activation(em[:], e1[:], AF.Relu,
                                             scale=-1.0, bias=1.0)
                        rl = FP.tile([128, HD], BF16, tag="rl",
                                     name=f"rl_{ic}_{hf}")
                        nc.scalar.activation(rl[:], pre[:], AF.Relu)
                        elu = FP.tile([128, HD], BF16, tag="elu",
                                      name=f"elu_{ic}_{hf}")
                        nc.vector.tensor_tensor(elu[:], rl[:], em[:],
                                                OP.subtract)
                        diff = FP.tile([128, HD], F32, tag="diff",
                                       name=f"df_{ic}_{hf}")
                        nc.gpsimd.tensor_tensor(diff[:], elu[:], fchunk[:],
                                                OP.subtract)
                        gd = FP.tile([128, HD], F32, tag="gd",
                                     name=f"gd_{ic}_{hf}")
                        nc.vector.tensor_tensor(gd[:],
                                                gate_sb[:, lo:lo + HD],
                                                diff[:], OP.mult)
                        outf = FP.tile([128, HD], F32, tag="outf",
                                       name=f"of_{ic}_{hf}")
                        nc.gpsimd.tensor_tensor(outf[:], fchunk[:], gd[:],
                                                OP.add)
                        nc.sync.dma_start(
                            out[ic * 128:(ic + 1) * 128,
                                hf * HD:(hf + 1) * HD], outf[:])

                def alloc_psa(rnd):
                    return [PSR.tile([128, 512], F32, tag="psa",
                                     name=f"psa_r{rnd}_{i}")
                            for i in range(IC)]

                psas = {}
                for rnd in range(4):
                    h0 = rnd * 2
                    build_head(h0, pool_heavy=False)
                    build_head(h0 + 1, pool_heavy=False)
                    psas[rnd] = alloc_psa(rnd)
                    mms_head(h0, psas[rnd])
                    mms_head(h0 + 1, psas[rnd])
                    evac_round(rnd, psas[rnd])
                    if rnd == 1:
                        emit_final(0)
                emit_final(1)

            EV.release()
            AB.release()
            PP.release()

    nc.compile()
    return nc
def _prep_shared(W1, W2, W3, w_src, w_dst, H_w, H_b, b):
    f32 = np.float32
    W1 = np.asarray(W1, f32)
    W1p = np.ascontiguousarray(
        W1.reshape(H, DC, 128, E).transpose(2, 1, 0, 3)
        .reshape(128, DC * H * E)).astype(ml_dtypes.bfloat16)
    W23 = np.stack([np.asarray(W2, f32).reshape(H, DC, 128, E),
                    np.asarray(W3, f32).reshape(H, DC, 128, E)], axis=2)
    # (h, dc, k, p, e) -> (p, dc, h, k, e)
    W23p = np.ascontiguousarray(
        W23.transpose(3, 1, 0, 2, 4)
        .reshape(128, DC * H * 2 * E)).astype(ml_dtypes.bfloat16)
    wsd = np.ascontiguousarray(
        np.concatenate([np.asarray(w_src, f32)[:, :, 0].T,
                        np.asarray(w_dst, f32)[:, :, 0].T], axis=1))  # [96, 16]
    Hwt = np.ascontiguousarray(np.asarray(H_w, f32).T
                               .reshape(DC, 128, D).transpose(1, 0, 2)
                               .reshape(128, DC * D)).astype(ml_dtypes.bfloat16)
    Hbr = np.ascontiguousarray(np.asarray(H_b, f32).reshape(1, D)).astype(ml_dtypes.bfloat16)
    # one-minus-eye chunks: omeye[p, jc*N + i] = 0 if i == jc*128+p else 1
    om = np.ones((128, JC * N), ml_dtypes.bfloat16)
    for jc in range(JC):
        idx = np.arange(128)
        om[idx, jc * N + jc * 128 + idx] = 0
    shared = {
        "W1p": W1p, "W23p": W23p, "wsd": wsd, "Hwt": Hwt, "Hb": Hbr,
        "omeye": om, "eye128": np.eye(128).astype(ml_dtypes.bfloat16),
        "ones_row": np.ones((1, 512), ml_dtypes.bfloat16),
        "ones128": np.ones((128, 128), f32),
    }
    b = np.asarray(b, f32)
    with_bias = bool(np.any(b != 0))
    if with_bias:
        shared["b3row"] = np.ascontiguousarray(np.tile(b / 3.0, H).reshape(1, D))
    return shared, with_bias


def _prep_core(feat, adjb, smb):
    f32 = np.float32
    feat = np.asarray(feat, f32)
    feat_nn = np.ascontiguousarray(
        feat.reshape(IC, 128, D).transpose(1, 0, 2).reshape(128, IC * D))
    featT = np.ascontiguousarray(
        feat.T.reshape(DC, 128, N).transpose(1, 0, 2)
        .reshape(128, DC * N)).astype(ml_dtypes.bfloat16)
    adjT = np.ascontiguousarray(
        adjb.T.reshape(JC, 128, N).transpose(1, 0, 2).reshape(128, JC * N))
    smT = np.ascontiguousarray(
        smb.T.reshape(JC, 128, N).transpose(1, 0, 2).reshape(128, JC * N))
    adjd = np.ascontiguousarray(
        np.diagonal(adjb).astype(f32).reshape(IC, 128).T)
    return {"feat_n": feat_nn, "featT": featT,
            "adjT_i": np.ascontiguousarray(adjT.astype(ml_dtypes.bfloat16)),
            "smT_i": np.ascontiguousarray(smT.astype(ml_dtypes.bfloat16)),
            "adjd": adjd}


def kernel(feat_in, adj, relation, s_mask, W1, W2, W3, b, w_src, w_dst,
           H_w, H_b, **_unused):
    global _CACHED
    shared, with_bias = _prep_shared(W1, W2, W3, w_src, w_dst, H_w, H_b, b)
    if _CACHED is None or _CACHED[1] != with_bias:
        _CACHED = (build_kernel(with_bias), with_bias)
    nc = _CACHED[0]

    feat_in = np.asarray(feat_in, np.float32)
    adj = np.asarray(adj, np.int32)
    s_mask = np.asarray(s_mask, np.int32)
    in_maps = []
    for c in range(B):
        m = dict(shared)
        m.update(_prep_core(feat_in[c], adj[c], s_mask[c]))
        in_maps.append(m)
    res = run_bass_kernel_spmd(nc, in_maps, core_ids=list(range(B)))
    outp = np.stack([res.results[c]["out"] for c in range(B)], axis=0)
    return outp.astype(np.float32)



# revision 3
# speedup vs baseline: 1.0602x; 1.0602x over previous
"""Trainium2 Bass kernel for nn_EnhancedAttentionLayer (GAT-style masked attention).

Data-parallel over batch: B=8 batch elements -> 8 NeuronCores, one each.
Params replicated. No collectives.

Math (per batch element, all heads on one core):
  h{1,2,3} = feat @ W{1,2,3}[h]      (per-head projections)
  t1 = tanh(h1); src = t1 @ w_src; dst = t1 @ w_dst
  attn1[i,j] = leaky_relu(src[i] + dst[j], 0.2)
  softmax-normalized masked attention, two mask variants (m2/m3)

Key trick (exact algebra): exp(leaky(x)) = exp(src_i)*exp(dst_j)*max(1,
exp(-0.8x)).  The exp(src_i) factor is a pure column scale in the [j, i]
layout, so it cancels between numerator and denominator of the normalized
attention.  Hence the unnormalized weight can be taken as
  q[j,i] = max(exp(-0.8*src_i) * exp(0.2*dst_j), exp(dst_j))
which is ONE fused tensor_scalar op per [128, N] tile (per-partition scalars
exp(0.2*dst_j), exp(dst_j)).  p2 = q*m2, p3 = q*m3; row sums via a ones
column in the matmul rhs.

Other structure:
  - p built TRANSPOSED [j, i] so it feeds matmul lhsT directly; masks m2T/m3T
    are computed HOST-side (0/1 bf16) - no on-device mask building.
  - activation-table thrash avoided: Exp/Tanh/Copy/Relu all live in the
    "exp_and_others" table; the 4 Sigmoid ops are grouped right after the
    gate pre-activations are ready (one table switch), and the ELU Exps
    switch back once.
  - H_b folded in via a broadcast row tensor (one matmul) + TT-add on evac
    instead of per-chunk bias matmuls.
"""

import numpy as np
import ml_dtypes

import concourse.bass as bass
import concourse.tile as tile
from concourse import bacc, mybir
from concourse.bass_utils import run_bass_kernel_spmd

F32 = mybir.dt.float32
BF16 = mybir.dt.bfloat16
AF = mybir.ActivationFunctionType
OP = mybir.AluOpType

B, N, D = 8, 512, 768
H, E = 8, 96
IC = N // 128   # 4 i-chunks (attention rows / output nodes)
JC = N // 128   # 4 j-chunks (attention cols / neighbor nodes)
DC = D // 128   # 6 contraction chunks
EPS = 1e-30

_CACHED = None


def build_kernel(with_bias: bool):
    nc = bacc.Bacc("TRN2", target_bir_lowering=False, debug=False, num_devices=B)

    # ---- per-core DRAM tensors (host pre-laid-out; see kernel() below) ----
    feat_n = nc.dram_tensor("feat_n", [128, IC * D], F32, kind="ExternalInput").ap()
    featT = nc.dram_tensor("featT", [128, DC * N], BF16, kind="ExternalInput").ap()
    m2T_i = nc.dram_tensor("m2T_i", [128, JC * N], BF16, kind="ExternalInput").ap()
    m3T_i = nc.dram_tensor("m3T_i", [128, JC * N], BF16, kind="ExternalInput").ap()
    adjd = nc.dram_tensor("adjd", [128, IC], F32, kind="ExternalInput").ap()
    W1p = nc.dram_tensor("W1p", [128, DC * H * E], BF16, kind="ExternalInput").ap()
    W23p = nc.dram_tensor("W23p", [128, DC * H * 2 * E], BF16, kind="ExternalInput").ap()
    wsd = nc.dram_tensor("wsd", [96, 16], F32, kind="ExternalInput").ap()
    Hwt = nc.dram_tensor("Hwt", [128, DC * D], BF16, kind="ExternalInput").ap()
    Hb = nc.dram_tensor("Hb", [1, D], BF16, kind="ExternalInput").ap()
    eye128 = nc.dram_tensor("eye128", [128, 128], BF16, kind="ExternalInput").ap()
    ones_row = nc.dram_tensor("ones_row", [1, 512], BF16, kind="ExternalInput").ap()
    ones128 = nc.dram_tensor("ones128", [128, 128], F32, kind="ExternalInput").ap()
    if with_bias:
        b3row = nc.dram_tensor("b3row", [1, D], F32, kind="ExternalOutput" if False else "ExternalInput").ap()
    out = nc.dram_tensor("out", [N, D], F32, kind="ExternalOutput").ap()

    with tile.TileContext(nc) as tc:
        with tc.tile_pool(name="persist", bufs=1) as P:
            # ------------- persistent SBUF tiles (live whole kernel) -------
            adjd_sb = P.tile([128, IC], F32, tag="adjd_sb")
            adjd3_sb = P.tile([128, IC], F32, tag="adjd3_sb")
            Hb_sb = P.tile([1, D], BF16, tag="Hb_sb")
            Hbrep = P.tile([128, D], F32, tag="Hbrep")                 # 3K
            eye_sb = P.tile([128, 128], BF16, tag="eye_sb")
            onesr_sb = P.tile([1, 512], BF16, tag="onesr_sb")
            ones_sb = P.tile([128, 128], F32, tag="ones_sb")
            m2T = P.tile([128, JC * N], BF16, tag="m2T")               # 4K
            m3T = P.tile([128, JC * N], BF16, tag="m3T")               # 4K
            h23 = P.tile([128, JC * H * 2 * 97], BF16, tag="h23")      # 12.1K
            em8s = P.tile([128, H * N], BF16, tag="em8s")              # 8K
            ed_sb = P.tile([128, H * JC], F32, tag="ed_sb")
            e02d_sb = P.tile([128, H * JC], F32, tag="e02d_sb")
            gpre = P.tile([128, IC * D], F32, tag="gpre")              # 12K
            gate_sb = P.tile([128, IC * D], BF16, tag="gate_sb")       # 6K
            feat_sb = P.tile([128, IC * D], F32, tag="feat_sb")        # 12K
            out23 = P.tile([128, IC * D], F32, tag="out23")            # 12K
            intra = P.tile([128, IC * D], F32, tag="intra")            # 12K
            b3_sb = P.tile([128, D], F32, tag="b3_sb") if with_bias else None

            h23r = h23[:].rearrange("p (jc h k eo) -> p jc h k eo",
                                    jc=JC, h=H, k=2, eo=97)
            intrar = intra[:].rearrange("p (i h e) -> p i h e", i=IC, h=H, e=96)

            PP = tc.alloc_tile_pool(name="ppool", bufs=4)
            QB = tc.alloc_tile_pool(name="qpool", bufs=3)
            EV = tc.alloc_tile_pool(name="evpool", bufs=3)
            with tc.tile_pool(name="wpool", bufs=1) as WP:
                W1p_sb = WP.tile([128, DC * H * E], BF16, tag="W1p_sb")      # 9K
                W23p_sb = WP.tile([128, DC * H * 2 * E], BF16, tag="W23p_sb")  # 18K
                Hwt_sb = WP.tile([128, DC * D], BF16, tag="Hwt_sb")          # 9K
                featT_bf = WP.tile([128, DC * N], BF16, tag="featT_bf")      # 6K
                TP = tc.alloc_tile_pool(name="tpool", bufs=1)
                t1T = TP.tile([96, H * N], BF16, tag="t1T")                  # 8K
                h1T = TP.tile([96, H * N], BF16, tag="h1T")                  # 8K
                wsr = TP.tile([96, H * 128], BF16, tag="wsr")                # 2K
                wsd_bf = TP.tile([96, 16], BF16, tag="wsd_bf")
                wsd_sb = TP.tile([96, 16], F32, tag="wsd_sb")

                # ---------------- input DMAs (priority order) ----------------
                # per-dc featT/W1p pairs so h1 matmuls can start early
                for dc in range(DC):
                    nc.sync.dma_start(featT_bf[:, dc * N:(dc + 1) * N],
                                      featT[:, dc * N:(dc + 1) * N])
                    nc.sync.dma_start(
                        W1p_sb[:, dc * 768:(dc + 1) * 768],
                        W1p[:, dc * 768:(dc + 1) * 768])
                nc.sync.dma_start(wsd_sb[:], wsd)
                nc.sync.dma_start(eye_sb[:], eye128)
                nc.sync.dma_start(ones_sb[:], ones128)
                nc.sync.dma_start(onesr_sb[:], ones_row)
                nc.sync.dma_start(adjd_sb[:], adjd)
                nc.sync.dma_start(Hb_sb[:], Hb)
                nc.sync.dma_start(W23p_sb[:], W23p)
                nc.sync.dma_start(Hwt_sb[:], Hwt)
                nc.sync.dma_start(m2T[:], m2T_i)
                nc.sync.dma_start(m3T[:], m3T_i)
                nc.sync.dma_start(feat_sb[:], feat_n)

                nc.vector.tensor_scalar(adjd3_sb[:], adjd_sb[:], 1.0 / 3.0,
                                        None, OP.mult)
                nc.vector.tensor_copy(wsd_bf[:], wsd_sb[:])

                # ---------------- h1 (transposed [e, i]) + tanh -------------
                with tc.tile_pool(name="ps1p", bufs=2, space="PSUM") as PS1:
                    for h in range(H):
                        ps1 = PS1.tile([96, 512], F32, tag="ps1")
                        for dc in range(DC):
                            nc.tensor.matmul(
                                ps1[:, :],
                                W1p_sb[:, dc * 768 + h * 96:
                                       dc * 768 + (h + 1) * 96],
                                featT_bf[:, dc * N:(dc + 1) * N],
                                start=(dc == 0), stop=(dc == DC - 1))
                        nc.scalar.activation(t1T[0:96, h * N:(h + 1) * N],
                                             ps1[:, :], AF.Tanh)
                        nc.vector.tensor_copy(h1T[0:96, h * N:(h + 1) * N],
                                              ps1[:, :])

                # ------- interleaved scores + h2/h3 + gate projections -------
                for h in range(H):
                    nc.vector.tensor_scalar(
                        wsr[0:96, h * 128:(h + 1) * 128], ones_sb[0:96, 0:128],
                        wsd_sb[0:96, h:h + 1], None, OP.mult)
                nc.vector.memset(h23r[:, :, :, :, 96:97], 1.0)        # ones cols

                # Hb broadcast row -> [128, D] f32 (one matmul + copy)
                with tc.tile_pool(name="hbp", bufs=1, space="PSUM") as HBP:
                    psb = HBP.tile([128, D], F32, tag="psb")
                    nc.tensor.matmul(psb[:, 0:512], onesr_sb[0:1, 0:128],
                                     Hb_sb[0:1, 0:512], start=True, stop=True)
                    nc.tensor.matmul(psb[:, 512:768], onesr_sb[0:1, 0:128],
                                     Hb_sb[0:1, 512:768], start=True, stop=True)
                    nc.scalar.copy(Hbrep[:], psb[:, :])

                with (
                    tc.tile_pool(name="pssp", bufs=1, space="PSUM") as PSS,
                    tc.tile_pool(name="ps23p", bufs=1, space="PSUM") as PS23,
                    tc.tile_pool(name="psgp", bufs=1, space="PSUM") as PSG,
                ):
                    for s in range(IC):
                        if s % 2 == 0:
                            psd = PSS.tile([128, 16], F32, tag="psd", bufs=1,
                                           name=f"psd_{s}")
                        for h in (2 * s, 2 * s + 1):
                            pss = PSS.tile([128, 512], F32, tag="pss")
                            nc.tensor.matmul(pss[:, :],
                                             wsr[0:96, h * 128:(h + 1) * 128],
                                             t1T[0:96, h * N:(h + 1) * N],
                                             start=True, stop=True)
                            nc.scalar.activation(em8s[:, h * N:(h + 1) * N],
                                                 pss[:, :], AF.Exp, scale=-0.8)
                            for jc in range(JC):
                                nc.tensor.matmul(
                                    psd[:, (h % 4) * JC + jc:
                                        (h % 4) * JC + jc + 1],
                                    t1T[0:96, h * N + jc * 128:
                                        h * N + (jc + 1) * 128],
                                    wsd_bf[0:96, 8 + h:9 + h],
                                    start=True, stop=True)
                        if s % 2 == 1:
                            g0 = (s - 1) * 2 * JC
                            nc.scalar.activation(
                                ed_sb[:, g0:g0 + 16], psd[:, :], AF.Exp)
                            nc.scalar.activation(
                                e02d_sb[:, g0:g0 + 16], psd[:, :], AF.Exp,
                                scale=0.2)
                        mc = s
                        ps23 = PS23.tile([128, 2048], F32, tag="ps23")
                        for dc in range(DC):
                            lhsT = featT_bf[:, dc * N + mc * 128:
                                            dc * N + (mc + 1) * 128]
                            for g in range(4):
                                nc.tensor.matmul(
                                    ps23[:, g * 512:g * 512 + 384],
                                    lhsT,
                                    W23p_sb[:, dc * 1536 + g * 384:
                                            dc * 1536 + (g + 1) * 384],
                                    start=(dc == 0), stop=(dc == DC - 1))
                        for g in range(4):
                            psrc = ps23[:, g * 512:g * 512 + 384].rearrange(
                                "p (h k e) -> p h k e", h=2, k=2, e=96)
                            eng = nc.scalar.copy if g % 2 == 0 else \
                                nc.vector.tensor_copy
                            eng(h23r[:, mc, 2 * g:2 * g + 2, :, 0:96], psrc)
                        psg = PSG.tile([128, 1024], F32, tag="psg")
                        for dc in range(DC):
                            lhsT = featT_bf[:, dc * N + mc * 128:
                                            dc * N + (mc + 1) * 128]
                            nc.tensor.matmul(psg[:, 0:512], lhsT,
                                             Hwt_sb[:, dc * D:dc * D + 512],
                                             start=(dc == 0), stop=(dc == DC - 1))
                            nc.tensor.matmul(psg[:, 512:768], lhsT,
                                             Hwt_sb[:, dc * D + 512:(dc + 1) * D],
                                             start=(dc == 0), stop=(dc == DC - 1))
                        nc.vector.tensor_tensor(
                            gpre[:, mc * D:(mc + 1) * D], psg[:, 0:768],
                            Hbrep[:], OP.add)

                # ------- grouped sigmoids (single table switch) -------
                for mc in range(IC):
                    nc.scalar.activation(gate_sb[:, mc * D:(mc + 1) * D],
                                         gpre[:, mc * D:(mc + 1) * D],
                                         AF.Sigmoid)

                # ---------------- h1 transpose + intra term -----------------
                with tc.tile_pool(name="pstp", bufs=2, space="PSUM") as PST:
                    for ic in range(IC):
                        pst = PST.tile([128, 1024], BF16, tag="pst")
                        for h in range(H):
                            nc.tensor.transpose(
                                pst[:, h * 128:h * 128 + 96],
                                h1T[0:96, h * N + ic * 128:h * N + (ic + 1) * 128],
                                eye_sb[0:96, 0:96])
                        pstr = pst[:].rearrange("p (u h o) -> p u h o",
                                                u=1, h=H, o=128)
                        nc.scalar.activation(
                            intrar[:, ic:ic + 1], pstr[:, :, :, 0:96],
                            AF.Copy, scale=adjd3_sb[:, ic:ic + 1])
                TP.release()

                if with_bias:
                    with tc.tile_pool(name="psbp", bufs=1, space="PSUM") as PSB:
                        psb2 = PSB.tile([128, D], F32, tag="psb2")
                        b3d = WP.tile([1, D], BF16, tag="b3d")
                        nc.gpsimd.dma_start(b3d[:], b3row)
                        nc.tensor.matmul(psb2[:, 0:512], onesr_sb[0:1, 0:128],
                                         b3d[0:1, 0:512], start=True, stop=True)
                        nc.tensor.matmul(psb2[:, 512:768], onesr_sb[0:1, 0:128],
                                         b3d[0:1, 512:768], start=True, stop=True)
                        nc.vector.tensor_copy(b3_sb[:], psb2[:, :])

            # ---------------- attention rounds ----------------
            with (
                tc.tile_pool(name="psrp", bufs=8, space="PSUM") as PSR,
                tc.tile_pool(name="fpool", bufs=3) as FP,
            ):
                p2t = {}
                p3t = {}

                def build_head(h):
                    q = QB.tile([128, JC * N], BF16, tag="q", name=f"q_h{h}")
                    p2 = PP.tile([128, JC * N], BF16, tag="p2",
                                 name=f"p2_h{h}")
                    p3 = PP.tile([128, JC * N], BF16, tag="p3",
                                 name=f"p3_h{h}")
                    p2t[h], p3t[h] = p2, p3
                    for jc in range(JC):
                        nc.vector.tensor_scalar(
                            q[:, jc * N:(jc + 1) * N],
                            em8s[:, h * N:(h + 1) * N],
                            e02d_sb[:, h * JC + jc:h * JC + jc + 1],
                            ed_sb[:, h * JC + jc:h * JC + jc + 1],
                            OP.mult, OP.max)
                    nc.vector.tensor_tensor(p2[:], q[:], m2T[:], OP.mult)
                    nc.gpsimd.tensor_tensor(p3[:], q[:], m3T[:], OP.mult)

                def mms_head(h, psa):
                    hh = h % 2
                    p2, p3 = p2t[h], p3t[h]
                    for ic in range(IC):
                        for k, p in ((0, p2), (1, p3)):
                            off = hh * 256 + k * 128
                            for jc in range(JC):
                                nc.tensor.matmul(
                                    psa[ic][:, off:off + 97],
                                    p[:, jc * N + ic * 128:
                                      jc * N + (ic + 1) * 128],
                                    h23[:, jc * 1552 + h * 194 + k * 97:
                                        jc * 1552 + h * 194 + k * 97 + 97],
                                    start=(jc == 0), stop=(jc == JC - 1))

                def evac_round(rnd, psa):
                    for ic in range(IC):
                        par = psa[ic][:].rearrange("p (s k o) -> p s k o",
                                                   s=2, k=2, o=128)
                        dden = EV.tile([128, 4], F32, tag="dden",
                                       name=f"dd_{rnd}_{ic}")
                        rcol = EV.tile([128, 4], F32, tag="rcol",
                                       name=f"rc_{rnd}_{ic}")
                        t23 = EV.tile([128, 384], F32, tag="t23",
                                      name=f"t23_{rnd}_{ic}")
                        ddenr = dden[:].rearrange("p (s k o) -> p s k o",
                                                  s=2, k=2, o=1)
                        nc.vector.tensor_scalar(
                            ddenr, par[:, :, :, 96:97], EPS, 3.0,
                            OP.add, OP.mult)
                        nc.vector.reciprocal(rcol[:], dden[:])
                        rbc = rcol[:].rearrange("p (s k) -> p s k", s=2, k=2) \
                                     .broadcast_to([128, 2, 2, 96])
                        t23r = t23[:].rearrange("p (s k e) -> p s k e",
                                                s=2, k=2, e=96)
                        nc.vector.tensor_tensor(t23r, par[:, :, :, 0:96],
                                                rbc, OP.mult)
                        nc.gpsimd.tensor_tensor(
                            out23[:, ic * D + rnd * 192:
                                  ic * D + rnd * 192 + 192]
                            .rearrange("p (s u e) -> p s u e", s=2, u=1, e=96),
                            t23r[:, :, 0:1, :], t23r[:, :, 1:2, :], OP.add)

                HD = 384
                def emit_final(hf):
                    for ic in range(IC):
                        lo = ic * D + hf * HD
                        pre = FP.tile([128, HD], F32, tag="pre",
                                      name=f"pre_{ic}_{hf}")
                        nc.vector.tensor_tensor(pre[:], out23[:, lo:lo + HD],
                                                intra[:, lo:lo + HD], OP.add)
                        if with_bias:
                            nc.gpsimd.tensor_tensor(
                                pre[:], pre[:],
                                b3_sb[:, hf * HD:(hf + 1) * HD], OP.add)
                        e1 = FP.tile([128, HD], F32, tag="e1",
                                     name=f"e1_{ic}_{hf}")
                        nc.scalar.activation(e1[:], pre[:], AF.Exp)
                        # em = relu(1 - e1) = -min(e1 - 1, 0)
                        em = FP.tile([128, HD], BF16, tag="em",
                                     name=f"em_{ic}_{hf}")
                        nc.scalar.activation(em[:], e1[:], AF.Relu,
                                             scale=-1.0, bias=1.0)
                        rl = FP.tile([128, HD], BF16, tag="rl",
                                     name=f"rl_{ic}_{hf}")
                        nc.scalar.activation(rl[:], pre[:], AF.Relu)
                        elu = FP.tile([128, HD], BF16, tag="elu",
                                      name=f"elu_{ic}_{hf}")
                        nc.vector.tensor_tensor(elu[:], rl[:], em[:],
                                                OP.subtract)
                        diff = FP.tile([128, HD], F32, tag="diff",
                                       name=f"df_{ic}_{hf}")
                        nc.gpsimd.tensor_tensor(diff[:], elu[:],
                                                feat_sb[:, lo:lo + HD],
                                                OP.subtract)
                        gd = FP.tile([128, HD], F32, tag="gd",
                                     name=f"gd_{ic}_{hf}")
                        nc.vector.tensor_tensor(gd[:],
                                                gate_sb[:, lo:lo + HD],
                                                diff[:], OP.mult)
                        outf = FP.tile([128, HD], F32, tag="outf",
                                       name=f"of_{ic}_{hf}")
                        nc.gpsimd.tensor_tensor(outf[:],
                                                feat_sb[:, lo:lo + HD],
                                                gd[:], OP.add)
                        nc.sync.dma_start(
                            out[ic * 128:(ic + 1) * 128,
                                hf * HD:(hf + 1) * HD], outf[:])

                def alloc_psa(rnd):
                    return [PSR.tile([128, 512], F32, tag="psa",
                                     name=f"psa_r{rnd}_{i}")
                            for i in range(IC)]

                psas = {}
                for rnd in range(4):
                    h0 = rnd * 2
                    build_head(h0)
                    build_head(h0 + 1)
                    psas[rnd] = alloc_psa(rnd)
                    mms_head(h0, psas[rnd])
                    mms_head(h0 + 1, psas[rnd])
                    evac_round(rnd, psas[rnd])
                    if rnd == 1:
                        emit_final(0)
                emit_final(1)

            EV.release()
            QB.release()
            PP.release()

    nc.compile()
    return nc


def _prep_shared(W1, W2, W3, w_src, w_dst, H_w, H_b, b):
    f32 = np.float32
    W1 = np.asarray(W1, f32)
    W1p = np.ascontiguousarray(
        W1.reshape(H, DC, 128, E).transpose(2, 1, 0, 3)
        .reshape(128, DC * H * E)).astype(ml_dtypes.bfloat16)
    W23 = np.stack([np.asarray(W2, f32).reshape(H, DC, 128, E),
                    np.asarray(W3, f32).reshape(H, DC, 128, E)], axis=2)
    # (h, dc, k, p, e) -> (p, dc, h, k, e)
    W23p = np.ascontiguousarray(
        W23.transpose(3, 1, 0, 2, 4)
        .reshape(128, DC * H * 2 * E)).astype(ml_dtypes.bfloat16)
    wsd = np.ascontiguousarray(
        np.concatenate([np.asarray(w_src, f32)[:, :, 0].T,
                        np.asarray(w_dst, f32)[:, :, 0].T], axis=1))  # [96, 16]
    Hwt = np.ascontiguousarray(np.asarray(H_w, f32).T
                               .reshape(DC, 128, D).transpose(1, 0, 2)
                               .reshape(128, DC * D)).astype(ml_dtypes.bfloat16)
    Hbr = np.ascontiguousarray(np.asarray(H_b, f32).reshape(1, D)).astype(ml_dtypes.bfloat16)
    shared = {
        "W1p": W1p, "W23p": W23p, "wsd": wsd, "Hwt": Hwt, "Hb": Hbr,
        "eye128": np.eye(128).astype(ml_dtypes.bfloat16),
        "ones_row": np.ones((1, 512), ml_dtypes.bfloat16),
        "ones128": np.ones((128, 128), f32),
    }
    b = np.asarray(b, f32)
    with_bias = bool(np.any(b != 0))
    if with_bias:
        shared["b3row"] = np.ascontiguousarray(np.tile(b / 3.0, H).reshape(1, D))
    return shared, with_bias


def _prep_core(feat, adjb, smb):
    f32 = np.float32
    feat = np.asarray(feat, f32)
    feat_nn = np.ascontiguousarray(
        feat.reshape(IC, 128, D).transpose(1, 0, 2).reshape(128, IC * D))
    featT = np.ascontiguousarray(
        feat.T.reshape(DC, 128, N).transpose(1, 0, 2)
        .reshape(128, DC * N)).astype(ml_dtypes.bfloat16)
    sm = smb.astype(f32)
    ad = adjb.astype(f32)
    m2 = sm.copy()
    np.fill_diagonal(m2, 0.0)            # sm * (1 - eye)
    m3 = ad * (1.0 - sm)                 # adj where different clause
    m2T = np.ascontiguousarray(
        m2.T.reshape(JC, 128, N).transpose(1, 0, 2).reshape(128, JC * N))
    m3T = np.ascontiguousarray(
        m3.T.reshape(JC, 128, N).transpose(1, 0, 2).reshape(128, JC * N))
    adjd = np.ascontiguousarray(
        np.diagonal(adjb).astype(f32).reshape(IC, 128).T)
    return {"feat_n": feat_nn, "featT": featT,
            "m2T_i": m2T.astype(ml_dtypes.bfloat16),
            "m3T_i": m3T.astype(ml_dtypes.bfloat16),
            "adjd": adjd}


def kernel(feat_in, adj, relation, s_mask, W1, W2, W3, b, w_src, w_dst,
           H_w, H_b, **_unused):
    global _CACHED
    shared, with_bias = _prep_shared(W1, W2, W3, w_src, w_dst, H_w, H_b, b)
    if _CACHED is None or _CACHED[1] != with_bias:
        _CACHED = (build_kernel(with_bias), with_bias)
    nc = _CACHED[0]

    feat_in = np.asarray(feat_in, np.float32)
    adj = np.asarray(adj, np.int32)
    s_mask = np.asarray(s_mask, np.int32)
    in_maps = []
    for c in range(B):
        m = dict(shared)
        m.update(_prep_core(feat_in[c], adj[c], s_mask[c]))
        in_maps.append(m)
    res = run_bass_kernel_spmd(nc, in_maps, core_ids=list(range(B)))
    outp = np.stack([res.results[c]["out"] for c in range(B)], axis=0)
    return outp.astype(np.float32)


# revision 7
# speedup vs baseline: 1.1211x; 1.0575x over previous
"""Trainium2 Bass kernel for nn_EnhancedAttentionLayer (GAT-style masked attention).

Data-parallel over batch: B=8 batch elements -> 8 NeuronCores, one each.
Params replicated. No collectives.

Key algebra:
  * exp(leaky(x)) = exp(src_i)*exp(dst_j)*max(1, exp(-0.8x)); the exp(src_i)
    column factor cancels in the normalized attention, so the unnormalized
    weight is q[j,i] = max(exp(-0.8*src_i)*exp(0.2*dst_j), exp(dst_j)) ==
    ONE fused tensor_scalar (per-partition scalars) per [128, N] tile.
  * sigmoid(x) = 0.5*(1+tanh(x/2)), and the gated residual
    g*elu + (1-g)*f = 0.5*[(elu+f) + tanh(x/2)*(elu-f)], so no Sigmoid
    activation is ever used -> every Act func (Tanh/Exp/Copy/Relu) lives in
    the single "exp_and_others" table: exactly one table load.
  * elu(x) = relu(x) + min(exp(x)-1, 0): the min/relu parts are DVE
    tensor_scalar ops in 4x mode.
  * masks m2T/m3T precomputed host-side; attention normalization divides
    happen on the (otherwise idle) Act engine via scale-Copies.
"""

import numpy as np
import ml_dtypes

import concourse.bass as bass
import concourse.tile as tile
from concourse import bacc, mybir
from concourse.bass_utils import run_bass_kernel_spmd

F32 = mybir.dt.float32
BF16 = mybir.dt.bfloat16
AF = mybir.ActivationFunctionType
OP = mybir.AluOpType

B, N, D = 8, 512, 768
H, E = 8, 96
IC = N // 128
JC = N // 128
DC = D // 128
EPS = 1e-30

_CACHED = None


def build_kernel(with_bias: bool):
    nc = bacc.Bacc("TRN2", target_bir_lowering=False, debug=False, num_devices=B)

    feat_n = nc.dram_tensor("feat_n", [128, IC * D], BF16, kind="ExternalInput").ap()
    featT = nc.dram_tensor("featT", [128, DC * N], BF16, kind="ExternalInput").ap()
    m2T_i = nc.dram_tensor("m2T_i", [128, JC * N], BF16, kind="ExternalInput").ap()
    m3T_i = nc.dram_tensor("m3T_i", [128, JC * N], BF16, kind="ExternalInput").ap()
    adjd = nc.dram_tensor("adjd", [128, IC], F32, kind="ExternalInput").ap()
    W1p = nc.dram_tensor("W1p", [128, DC * H * E], BF16, kind="ExternalInput").ap()
    W23p = nc.dram_tensor("W23p", [128, DC * H * 2 * E], BF16, kind="ExternalInput").ap()
    wsd = nc.dram_tensor("wsd", [96, 16], F32, kind="ExternalInput").ap()
    Hwt = nc.dram_tensor("Hwt", [128, DC * D], BF16, kind="ExternalInput").ap()
    Hb = nc.dram_tensor("Hb", [1, D], BF16, kind="ExternalInput").ap()
    eye128 = nc.dram_tensor("eye128", [128, 128], BF16, kind="ExternalInput").ap()
    ones_row = nc.dram_tensor("ones_row", [1, 512], BF16, kind="ExternalInput").ap()
    ones128 = nc.dram_tensor("ones128", [128, 128], F32, kind="ExternalInput").ap()
    if with_bias:
        b3row = nc.dram_tensor("b3row", [1, D], BF16, kind="ExternalInput").ap()
    out = nc.dram_tensor("out", [N, D], BF16, kind="ExternalOutput").ap()

    with tile.TileContext(nc) as tc:
        with tc.tile_pool(name="persist", bufs=1) as P:
            adjd_sb = P.tile([128, IC], F32, tag="adjd_sb")
            adjd3_sb = P.tile([128, IC], F32, tag="adjd3_sb")
            Hb_sb = P.tile([1, D], BF16, tag="Hb_sb")
            eye_sb = P.tile([128, 128], BF16, tag="eye_sb")
            onesr_sb = P.tile([1, 512], BF16, tag="onesr_sb")
            ones_sb = P.tile([128, 128], F32, tag="ones_sb")
            m2T = P.tile([128, JC * N], BF16, tag="m2T")               # 4K
            m3T = P.tile([128, JC * N], BF16, tag="m3T")               # 4K
            h23 = P.tile([128, JC * H * 2 * 97], BF16, tag="h23")      # 12.1K
            em8s = P.tile([128, H * N], BF16, tag="em8s")              # 8K
            ed_sb = P.tile([128, H * JC], F32, tag="ed_sb")
            e02d_sb = P.tile([128, H * JC], F32, tag="e02d_sb")
            th_sb = P.tile([128, IC * D], BF16, tag="th_sb")           # 6K
            thh_sb = P.tile([128, IC * D], BF16, tag="thh_sb")         # 6K
            feat_sb = P.tile([128, IC * D], BF16, tag="feat_sb")       # 6K
            out23 = P.tile([128, IC * D], BF16, tag="out23")           # 6K
            intra = P.tile([128, IC * D], BF16, tag="intra")           # 6K
            b3_sb = P.tile([128, D], F32, tag="b3_sb") if with_bias else None

            h23r = h23[:].rearrange("p (jc h k eo) -> p jc h k eo",
                                    jc=JC, h=H, k=2, eo=97)
            intrar = intra[:].rearrange("p (i h e) -> p i h e", i=IC, h=H, e=96)

            PP = tc.alloc_tile_pool(name="ppool", bufs=4)
            QB = tc.alloc_tile_pool(name="qpool", bufs=3)
            EV = tc.alloc_tile_pool(name="evpool", bufs=4)
            with tc.tile_pool(name="wpool", bufs=1) as WP:
                W1p_sb = WP.tile([128, DC * H * E], BF16, tag="W1p_sb")      # 9K
                W23p_sb = WP.tile([128, DC * H * 2 * E], BF16, tag="W23p_sb")  # 18K
                Hwt_sb = WP.tile([128, DC * D], BF16, tag="Hwt_sb")          # 9K
                featT_bf = WP.tile([128, DC * N], BF16, tag="featT_bf")      # 6K
                TP = tc.alloc_tile_pool(name="tpool", bufs=1)
                t1T = TP.tile([96, H * N], BF16, tag="t1T")                  # 8K
                h1T = TP.tile([96, H * N], BF16, tag="h1T")                  # 8K
                wsr = TP.tile([96, H * 128], BF16, tag="wsr")                # 2K
                wsd_bf = TP.tile([96, 16], BF16, tag="wsd_bf")
                wsd_sb = TP.tile([96, 16], F32, tag="wsd_sb")

                # ---------------- input DMAs (priority order) ----------------
                for dc in range(DC):
                    nc.sync.dma_start(featT_bf[:, dc * N:(dc + 1) * N],
                                      featT[:, dc * N:(dc + 1) * N])
                    nc.sync.dma_start(
                        W1p_sb[:, dc * 768:(dc + 1) * 768],
                        W1p[:, dc * 768:(dc + 1) * 768])
                nc.sync.dma_start(wsd_sb[:], wsd)
                nc.sync.dma_start(eye_sb[:], eye128)
                nc.sync.dma_start(ones_sb[:], ones128)
                nc.sync.dma_start(onesr_sb[:], ones_row)
                nc.sync.dma_start(adjd_sb[:], adjd)
                nc.sync.dma_start(Hb_sb[:], Hb)
                nc.sync.dma_start(W23p_sb[:], W23p)
                nc.sync.dma_start(m2T[:], m2T_i)
                nc.sync.dma_start(m3T[:], m3T_i)
                nc.sync.dma_start(Hwt_sb[:], Hwt)
                nc.sync.dma_start(feat_sb[:], feat_n)

                nc.vector.tensor_scalar(adjd3_sb[:], adjd_sb[:], 1.0 / 3.0,
                                        None, OP.mult)
                nc.vector.tensor_copy(wsd_bf[:], wsd_sb[:])

                # ---------------- h1 (transposed [e, i]) + tanh -------------
                with tc.tile_pool(name="ps1p", bufs=2, space="PSUM") as PS1:
                    for h in range(H):
                        ps1 = PS1.tile([96, 512], F32, tag="ps1")
                        for dc in range(DC):
                            nc.tensor.matmul(
                                ps1[:, :],
                                W1p_sb[:, dc * 768 + h * 96:
                                       dc * 768 + (h + 1) * 96],
                                featT_bf[:, dc * N:(dc + 1) * N],
                                start=(dc == 0), stop=(dc == DC - 1))
                        nc.scalar.activation(t1T[0:96, h * N:(h + 1) * N],
                                             ps1[:, :], AF.Tanh)
                        nc.vector.tensor_copy(h1T[0:96, h * N:(h + 1) * N],
                                              ps1[:, :])

                # ------- interleaved scores + h2/h3 projections -------
                for h in range(H):
                    nc.vector.tensor_scalar(
                        wsr[0:96, h * 128:(h + 1) * 128], ones_sb[0:96, 0:128],
                        wsd_sb[0:96, h:h + 1], None, OP.mult)
                nc.vector.memset(h23r[:, :, :, :, 96:97], 1.0)        # ones cols

                with (
                    tc.tile_pool(name="pssp", bufs=1, space="PSUM") as PSS,
                    tc.tile_pool(name="ps23p", bufs=1, space="PSUM") as PS23,
                ):
                    for s in range(IC):
                        if s % 2 == 0:
                            psd = PSS.tile([128, 16], F32, tag="psd", bufs=1,
                                           name=f"psd_{s}")
                        for h in (2 * s, 2 * s + 1):
                            pss = PSS.tile([128, 512], F32, tag="pss")
                            nc.tensor.matmul(pss[:, :],
                                             wsr[0:96, h * 128:(h + 1) * 128],
                                             t1T[0:96, h * N:(h + 1) * N],
                                             start=True, stop=True)
                            nc.scalar.activation(em8s[:, h * N:(h + 1) * N],
                                                 pss[:, :], AF.Exp, scale=-0.8)
                            for jc in range(JC):
                                nc.tensor.matmul(
                                    psd[:, (h % 4) * JC + jc:
                                        (h % 4) * JC + jc + 1],
                                    t1T[0:96, h * N + jc * 128:
                                        h * N + (jc + 1) * 128],
                                    wsd_bf[0:96, 8 + h:9 + h],
                                    start=True, stop=True)
                        if s % 2 == 1:
                            g0 = (s - 1) * 2 * JC
                            nc.scalar.activation(
                                ed_sb[:, g0:g0 + 16], psd[:, :], AF.Exp)
                            nc.scalar.activation(
                                e02d_sb[:, g0:g0 + 16], psd[:, :], AF.Exp,
                                scale=0.2)
                        mc = s
                        ps23 = PS23.tile([128, 2048], F32, tag="ps23")
                        for dc in range(DC):
                            lhsT = featT_bf[:, dc * N + mc * 128:
                                            dc * N + (mc + 1) * 128]
                            for g in range(4):
                                nc.tensor.matmul(
                                    ps23[:, g * 512:g * 512 + 384],
                                    lhsT,
                                    W23p_sb[:, dc * 1536 + g * 384:
                                            dc * 1536 + (g + 1) * 384],
                                    start=(dc == 0), stop=(dc == DC - 1))
                        for g in range(4):
                            psrc = ps23[:, g * 512:g * 512 + 384].rearrange(
                                "p (h k e) -> p h k e", h=2, k=2, e=96)
                            nc.scalar.copy(h23r[:, mc, 2 * g:2 * g + 2, :, 0:96],
                                           psrc)

                # -------- gate block: x = feat@Hw + Hb; th = tanh(x/2) ------
                with tc.tile_pool(name="psgp", bufs=2, space="PSUM") as PSG:
                    for mc in range(IC):
                        psg = PSG.tile([128, 1024], F32, tag="psg")
                        for dc in range(DC):
                            lhsT = featT_bf[:, dc * N + mc * 128:
                                            dc * N + (mc + 1) * 128]
                            nc.tensor.matmul(psg[:, 0:512], lhsT,
                                             Hwt_sb[:, dc * D:dc * D + 512],
                                             start=(dc == 0), stop=False)
                            nc.tensor.matmul(psg[:, 512:768], lhsT,
                                             Hwt_sb[:, dc * D + 512:(dc + 1) * D],
                                             start=(dc == 0), stop=False)
                        nc.tensor.matmul(psg[:, 0:512], onesr_sb[0:1, 0:128],
                                         Hb_sb[0:1, 0:512],
                                         start=False, stop=True)
                        nc.tensor.matmul(psg[:, 512:768], onesr_sb[0:1, 0:128],
                                         Hb_sb[0:1, 512:768],
                                         start=False, stop=True)
                        nc.scalar.activation(th_sb[:, mc * D:(mc + 1) * D],
                                             psg[:, 0:768], AF.Tanh, scale=0.5)
                        nc.vector.tensor_scalar(
                            thh_sb[:, mc * D:(mc + 1) * D],
                            th_sb[:, mc * D:(mc + 1) * D], 0.5, None, OP.mult)

                # ---------------- h1 transpose + intra term -----------------
                with tc.tile_pool(name="pstp", bufs=2, space="PSUM") as PST:
                    for ic in range(IC):
                        pst = PST.tile([128, 1024], BF16, tag="pst")
                        for h in range(H):
                            nc.tensor.transpose(
                                pst[:, h * 128:h * 128 + 96],
                                h1T[0:96, h * N + ic * 128:h * N + (ic + 1) * 128],
                                eye_sb[0:96, 0:96])
                        pstr = pst[:].rearrange("p (u h o) -> p u h o",
                                                u=1, h=H, o=128)
                        nc.vector.tensor_scalar(
                            intrar[:, ic:ic + 1], pstr[:, :, :, 0:96],
                            adjd3_sb[:, ic:ic + 1], None, OP.mult)
                TP.release()

                if with_bias:
                    with tc.tile_pool(name="psbp", bufs=1, space="PSUM") as PSB:
                        psb2 = PSB.tile([128, D], F32, tag="psb2")
                        b3d = WP.tile([1, D], BF16, tag="b3d")
                        nc.gpsimd.dma_start(b3d[:], b3row)
                        nc.tensor.matmul(psb2[:, 0:512], onesr_sb[0:1, 0:128],
                                         b3d[0:1, 0:512], start=True, stop=True)
                        nc.tensor.matmul(psb2[:, 512:768], onesr_sb[0:1, 0:128],
                                         b3d[0:1, 512:768], start=True, stop=True)
                        nc.vector.tensor_copy(b3_sb[:], psb2[:, :])

            # ---------------- attention rounds ----------------
            with (
                tc.tile_pool(name="psrp", bufs=8, space="PSUM") as PSR,
                tc.tile_pool(name="fpool", bufs=3) as FP,
            ):
                p2t = {}
                p3t = {}

                def build_head(h):
                    q = QB.tile([128, JC * N], BF16, tag="q", name=f"q_h{h}")
                    p2 = PP.tile([128, JC * N], BF16, tag="p2",
                                 name=f"p2_h{h}")
                    p3 = PP.tile([128, JC * N], BF16, tag="p3",
                                 name=f"p3_h{h}")
                    p2t[h], p3t[h] = p2, p3
                    for jc in range(JC):
                        nc.vector.tensor_scalar(
                            q[:, jc * N:(jc + 1) * N],
                            em8s[:, h * N:(h + 1) * N],
                            e02d_sb[:, h * JC + jc:h * JC + jc + 1],
                            ed_sb[:, h * JC + jc:h * JC + jc + 1],
                            OP.mult, OP.max)
                    nc.vector.tensor_tensor(p2[:], q[:], m2T[:], OP.mult)
                    # p3 split between DVE (2 jc) and Pool (2 jc)
                    nc.vector.tensor_tensor(p3[:, 0:N * 2], q[:, 0:N * 2],
                                            m3T[:, 0:N * 2], OP.mult)
                    nc.gpsimd.tensor_tensor(p3[:, N * 2:N * 4], q[:, N * 2:N * 4],
                                            m3T[:, N * 2:N * 4], OP.mult)

                def mms_head(h, psa):
                    hh = h % 2
                    p2, p3 = p2t[h], p3t[h]
                    for ic in range(IC):
                        for k, p in ((0, p2), (1, p3)):
                            off = hh * 256 + k * 128
                            for jc in range(JC):
                                nc.tensor.matmul(
                                    psa[ic][:, off:off + 97],
                                    p[:, jc * N + ic * 128:
                                      jc * N + (ic + 1) * 128],
                                    h23[:, jc * 1552 + h * 194 + k * 97:
                                        jc * 1552 + h * 194 + k * 97 + 97],
                                    start=(jc == 0), stop=(jc == JC - 1))

                def evac_round(rnd, psa):
                    # psa[ic][:, (s, k, 128)]: cols 0:96 numerators, 96 = den.
                    # dden = (den + eps)*3 on Act (Copy w/ scale+bias);
                    # rcol = 1/dden on DVE; numerators scaled by rcol via Act
                    # scale-Copies; k=1 added into out23 by DVE/Pool.
                    for ic in range(IC):
                        par = psa[ic][:].rearrange("p (s k o) -> p s k o",
                                                   s=2, k=2, o=128)
                        dden = EV.tile([128, 4], F32, tag="dden",
                                       name=f"dd_{rnd}_{ic}")
                        rcol = EV.tile([128, 4], F32, tag="rcol",
                                       name=f"rc_{rnd}_{ic}")
                        t3t = EV.tile([128, 192], BF16, tag="t3t",
                                      name=f"t3_{rnd}_{ic}")
                        ddenr = dden[:].rearrange("p (s k o) -> p s k o",
                                                  s=2, k=2, o=1)
                        nc.scalar.activation(ddenr, par[:, :, :, 96:97],
                                             AF.Copy, scale=3.0, bias=3.0 * EPS)
                        nc.vector.reciprocal(rcol[:], dden[:])
                        t3r = t3t[:].rearrange("p (s e) -> p s e", s=2, e=96)
                        o23 = out23[:, ic * D + rnd * 192:ic * D + rnd * 192
                                    + 192].rearrange("p (s e) -> p s e",
                                                     s=2, e=96)
                        for s in range(2):
                            nc.scalar.activation(
                                o23[:, s:s + 1], par[:, s:s + 1, 0, 0:96],
                                AF.Copy, scale=rcol[:, 2 * s:2 * s + 1])
                            nc.scalar.activation(
                                t3r[:, s:s + 1], par[:, s:s + 1, 1, 0:96],
                                AF.Copy, scale=rcol[:, 2 * s + 1:2 * s + 2])
                        eng = nc.vector.tensor_tensor if ic % 2 == 0 else \
                            nc.gpsimd.tensor_tensor
                        eng(out23[:, ic * D + rnd * 192:
                                  ic * D + rnd * 192 + 192],
                            out23[:, ic * D + rnd * 192:
                                  ic * D + rnd * 192 + 192],
                            t3t[:], OP.add)

                HD = 384
                def emit_final(hf):
                    # out = 0.5*[(elu+f) + th*(elu-f)]  (th = tanh(gate/2))
                    for ic in range(IC):
                        lo = ic * D + hf * HD
                        pre = FP.tile([128, HD], BF16, tag="pre",
                                      name=f"pre_{ic}_{hf}")
                        nc.vector.tensor_tensor(pre[:], out23[:, lo:lo + HD],
                                                intra[:, lo:lo + HD], OP.add)
                        if with_bias:
                            nc.gpsimd.tensor_tensor(
                                pre[:], pre[:],
                                b3_sb[:, hf * HD:(hf + 1) * HD], OP.add)
                        e1 = FP.tile([128, HD], BF16, tag="e1",
                                     name=f"e1_{ic}_{hf}")
                        nc.scalar.activation(e1[:], pre[:], AF.Exp)
                        em = FP.tile([128, HD], BF16, tag="em",
                                     name=f"em_{ic}_{hf}")
                        nc.vector.tensor_scalar(em[:], e1[:], -1.0, 0.0,
                                                OP.add, OP.min)
                        rl = FP.tile([128, HD], BF16, tag="rl",
                                     name=f"rl_{ic}_{hf}")
                        nc.vector.tensor_scalar(rl[:], pre[:], 0.0, None,
                                                OP.max)
                        elu = FP.tile([128, HD], BF16, tag="elu",
                                      name=f"elu_{ic}_{hf}")
                        nc.vector.tensor_tensor(elu[:], rl[:], em[:], OP.add)
                        dd = FP.tile([128, HD], BF16, tag="dd",
                                     name=f"dd_{ic}_{hf}")
                        nc.vector.tensor_tensor(dd[:], elu[:],
                                                feat_sb[:, lo:lo + HD],
                                                OP.subtract)
                        ss = FP.tile([128, HD], BF16, tag="ss",
                                     name=f"ss_{ic}_{hf}")
                        nc.vector.tensor_tensor(ss[:], elu[:],
                                                feat_sb[:, lo:lo + HD],
                                                OP.add)
                        ss2 = FP.tile([128, HD], BF16, tag="ss2",
                                      name=f"s2_{ic}_{hf}")
                        nc.vector.tensor_scalar(ss2[:], ss[:], 0.5, None,
                                                OP.mult)
                        ww = FP.tile([128, HD], BF16, tag="ww",
                                     name=f"ww_{ic}_{hf}")
                        nc.gpsimd.tensor_tensor(ww[:], thh_sb[:, lo:lo + HD],
                                                dd[:], OP.mult)
                        outf = FP.tile([128, HD], BF16, tag="outf",
                                       name=f"of_{ic}_{hf}")
                        nc.gpsimd.tensor_tensor(outf[:], ss2[:], ww[:], OP.add)
                        nc.sync.dma_start(
                            out[ic * 128:(ic + 1) * 128,
                                hf * HD:(hf + 1) * HD], outf[:])

                def alloc_psa(rnd):
                    return [PSR.tile([128, 512], F32, tag="psa",
                                     name=f"psa_r{rnd}_{i}")
                            for i in range(IC)]

                psas = {}
                for rnd in range(4):
                    h0 = rnd * 2
                    build_head(h0)
                    build_head(h0 + 1)
                    psas[rnd] = alloc_psa(rnd)
                    mms_head(h0, psas[rnd])
                    mms_head(h0 + 1, psas[rnd])
                    evac_round(rnd, psas[rnd])
                    if rnd == 1:
                        emit_final(0)
                emit_final(1)

            EV.release()
            QB.release()
            PP.release()

    nc.compile()
    return nc


def _prep_shared(W1, W2, W3, w_src, w_dst, H_w, H_b, b):
    f32 = np.float32
    W1 = np.asarray(W1, f32)
    W1p = np.ascontiguousarray(
        W1.reshape(H, DC, 128, E).transpose(2, 1, 0, 3)
        .reshape(128, DC * H * E)).astype(ml_dtypes.bfloat16)
    W23 = np.stack([np.asarray(W2, f32).reshape(H, DC, 128, E),
                    np.asarray(W3, f32).reshape(H, DC, 128, E)], axis=2)
    W23p = np.ascontiguousarray(
        W23.transpose(3, 1, 0, 2, 4)
        .reshape(128, DC * H * 2 * E)).astype(ml_dtypes.bfloat16)
    wsd = np.ascontiguousarray(
        np.concatenate([np.asarray(w_src, f32)[:, :, 0].T,
                        np.asarray(w_dst, f32)[:, :, 0].T], axis=1))  # [96, 16]
    Hwt = np.ascontiguousarray(np.asarray(H_w, f32).T
                               .reshape(DC, 128, D).transpose(1, 0, 2)
                               .reshape(128, DC * D)).astype(ml_dtypes.bfloat16)
    Hbr = np.ascontiguousarray(np.asarray(H_b, f32).reshape(1, D)).astype(ml_dtypes.bfloat16)
    shared = {
        "W1p": W1p, "W23p": W23p, "wsd": wsd, "Hwt": Hwt, "Hb": Hbr,
        "eye128": np.eye(128).astype(ml_dtypes.bfloat16),
        "ones_row": np.ones((1, 512), ml_dtypes.bfloat16),
        "ones128": np.ones((128, 128), f32),
    }
    b = np.asarray(b, f32)
    with_bias = bool(np.any(b != 0))
    if with_bias:
        shared["b3row"] = np.ascontiguousarray(
            np.tile(b / 3.0, H).reshape(1, D)).astype(ml_dtypes.bfloat16)
    return shared, with_bias


def _prep_core(feat, adjb, smb):
    f32 = np.float32
    feat = np.asarray(feat, f32)
    feat_nn = np.ascontiguousarray(
        feat.reshape(IC, 128, D).transpose(1, 0, 2)
        .reshape(128, IC * D)).astype(ml_dtypes.bfloat16)
    featT = np.ascontiguousarray(
        feat.T.reshape(DC, 128, N).transpose(1, 0, 2)
        .reshape(128, DC * N)).astype(ml_dtypes.bfloat16)
    sm = smb.astype(f32)
    ad = adjb.astype(f32)
    m2 = sm.copy()
    np.fill_diagonal(m2, 0.0)            # sm * (1 - eye)
    m3 = ad * (1.0 - sm)                 # adj where different clause
    m2T = np.ascontiguousarray(
        m2.T.reshape(JC, 128, N).transpose(1, 0, 2).reshape(128, JC * N))
    m3T = np.ascontiguousarray(
        m3.T.reshape(JC, 128, N).transpose(1, 0, 2).reshape(128, JC * N))
    adjd = np.ascontiguousarray(
        np.diagonal(adjb).astype(f32).reshape(IC, 128).T)
    return {"feat_n": feat_nn, "featT": featT,
            "m2T_i": m2T.astype(ml_dtypes.bfloat16),
            "m3T_i": m3T.astype(ml_dtypes.bfloat16),
            "adjd": adjd}


def kernel(feat_in, adj, relation, s_mask, W1, W2, W3, b, w_src, w_dst,
           H_w, H_b, **_unused):
    global _CACHED
    shared, with_bias = _prep_shared(W1, W2, W3, w_src, w_dst, H_w, H_b, b)
    if _CACHED is None or _CACHED[1] != with_bias:
        _CACHED = (build_kernel(with_bias), with_bias)
    nc = _CACHED[0]

    feat_in = np.asarray(feat_in, np.float32)
    adj = np.asarray(adj, np.int32)
    s_mask = np.asarray(s_mask, np.int32)
    in_maps = []
    for c in range(B):
        m = dict(shared)
        m.update(_prep_core(feat_in[c], adj[c], s_mask[c]))
        in_maps.append(m)
    res = run_bass_kernel_spmd(nc, in_maps, core_ids=list(range(B)))
    outp = np.stack([np.asarray(res.results[c]["out"]).astype(np.float32)
                     for c in range(B)], axis=0)
    return outp
